# revision 10
# baseline (speedup 1.0000x reference)
"""Trainium2 Bass kernel for nn_Interpolator: zero-stuff upsample x8 + 128-tap FIR (SAME) + x8 gain.

Polyphase formulation: with m indexing 64-sample rows of x and n = 8*q' + r in [0, 512),
    y[512*m + n] = sum_{k=0}^{78} T4[k, m] * H4[k, n]
where T4[k, m] = x[64*m + k - 7] (zero-padded) and
    H4[k, 8*q'+r] = 8 * h[(7-r) + 8*(k-q')]  for 0 <= k-q' <= 15, else 0.

Per core (8 cores, batch-parallel): 16 signals (8 batch rows x {real, imag}).
Per signal: load host-padded x as [128, 271] fp16 (partition p = x[256p-7 : 256p+264]),
PE-transpose four 79-column slices into one PSUM bank, bulk-copy to T4 [79, 512] fp16
(columns interleaved m = 4p + c), then 4 matmuls lhsT=T4[:, 128t:+128], rhs=H4 -> PSUM
fp32 [128, 512], copy to SBUF, DMA out contiguously (partition i of tile t holds
y[65536t + 512i : +512]).  Matmul stage runs one signal behind the transpose stage so
the PE never waits on the PSUM->SBUF interleave copy.
"""

import numpy as np

import concourse.bass as bass
import concourse.tile as tile
from concourse import bacc, mybir
from concourse.bass_utils import run_bass_kernel_spmd

B = 64
N = 32768
FACTOR = 8
NOUT = N * FACTOR  # 262144
N_CORES = 8
ROWS_PER_CORE = B // N_CORES  # 8
SIGS = 2 * ROWS_PER_CORE  # 16 signals per core (real rows then imag rows)
K = 79  # contraction window length
XCOLS = 271  # 256 + 15 halo
NPAD = 32784  # 7 leading zeros + N + 8 trailing zeros + 1 spare (host-padded)
TILES = 4  # out tiles per signal, each [128 m-rows, 512 samples]

_F16 = mybir.dt.float16
_F32 = mybir.dt.float32

_NC_CACHE = {}


def _build_nc():
    nc = bacc.Bacc(
        "TRN2",
        target_bir_lowering=False,
        debug=False,
        enable_asserts=False,
        num_devices=N_CORES,
    )
    x = nc.dram_tensor("x", [SIGS, NPAD], _F16, kind="ExternalInput")
    h4 = nc.dram_tensor("h4", [K, 512], _F16, kind="ExternalInput")
    ident = nc.dram_tensor("ident", [128, 128], _F16, kind="ExternalInput")
    y = nc.dram_tensor("y", [SIGS, NOUT], _F32, kind="ExternalOutput")

    with tile.TileContext(nc) as tc:
        with (
            tc.tile_pool(name="consts", bufs=1) as consts,
            tc.tile_pool(name="xpool", bufs=2) as xpool,
            tc.tile_pool(name="t4pool", bufs=2) as t4pool,
            tc.tile_pool(name="opool", bufs=2) as opool,
            tc.tile_pool(name="pt", bufs=2, space="PSUM") as pt_pool,
            tc.tile_pool(name="po", bufs=2, space="PSUM") as po_pool,
        ):
            h4_sb = consts.tile([K, 512], _F16)
            nc.sync.dma_start(out=h4_sb, in_=h4.ap())
            ident_sb = consts.tile([128, 128], _F16)
            nc.sync.dma_start(out=ident_sb, in_=ident.ap())

            t4_tiles = [None] * SIGS

            def stage_a(sig):
                """Load + transpose + interleave-copy into T4."""
                xoff = sig * NPAD
                # partition p holds x_pad[256p : 256p + 271] = x[256p - 7 : 256p + 264]
                X = xpool.tile([128, XCOLS], _F16)
                nc.sync.dma_start(
                    out=X[:, :],
                    in_=bass.AP(tensor=x, offset=xoff, ap=[[256, 128], [1, XCOLS]]),
                )
                # All 4 transposes write one PSUM bank, one bulk copy out.
                # T4[k, 4p + c] = pt4[k, c, p] = X[p, 64c + k] = x[256p + 64c + k - 7]
                T4 = t4pool.tile([K, 512], _F16)
                T4i = T4[:, :].rearrange("k (p four) -> k four p", four=4)
                pt4 = pt_pool.tile([K, 4, 128], _F16)
                for c in range(4):
                    nc.tensor.transpose(
                        pt4[:, c, :], X[:, 64 * c : 64 * c + K], ident_sb
                    )
                if sig % 2 == 0:
                    nc.vector.tensor_copy(out=T4i, in_=pt4[:, :, :])
                else:
                    nc.scalar.copy(out=T4i, in_=pt4[:, :, :])
                t4_tiles[sig] = T4

            def stage_b(sig):
                """Matmuls + copy-out + store."""
                T4 = t4_tiles[sig]
                out_sb = opool.tile([128, TILES * 512], _F32)
                for half in range(2):
                    po = po_pool.tile([128, 1024], _F32)
                    for s in range(2):
                        t = 2 * half + s
                        nc.tensor.matmul(
                            po[:, 512 * s : 512 * (s + 1)],
                            T4[:, 128 * t : 128 * (t + 1)],
                            h4_sb[:, :],
                            start=True,
                            stop=True,
                        )
                    if half == 0:
                        nc.scalar.copy(out=out_sb[:, 0:1024], in_=po)
                    else:
                        nc.vector.tensor_copy(out=out_sb[:, 1024:2048], in_=po)
                # partition i, free (t, n) -> y[sig, 65536t + 512i + n]
                nc.gpsimd.dma_start(
                    out=bass.AP(
                        tensor=y,
                        offset=sig * NOUT,
                        ap=[[512, 128], [65536, TILES], [1, 512]],
                    ),
                    in_=out_sb[:, :],
                )

            for sig in range(SIGS):
                stage_a(sig)
                if sig >= 1:
                    stage_b(sig - 1)
            stage_b(SIGS - 1)

    nc.compile()
    return nc


def _get_nc():
    if "nc" not in _NC_CACHE:
        _NC_CACHE["nc"] = _build_nc()
    return _NC_CACHE["nc"]


def _build_h4(h):
    h4 = np.zeros((K, 512), np.float32)
    qp = np.arange(64)
    for t in range(16):
        for r in range(8):
            h4[qp + t, 8 * qp + r] = FACTOR * h[(7 - r) + 8 * t]
    return h4


def _run(x_real, x_imag, fir_filter, trace=False):
    h4 = _build_h4(np.asarray(fir_filter, np.float32)).astype(np.float16)
    ident = np.eye(128, dtype=np.float16)
    in_maps = []
    for c in range(N_CORES):
        rows = slice(c * ROWS_PER_CORE, (c + 1) * ROWS_PER_CORE)
        shard = np.zeros((SIGS, NPAD), np.float16)
        shard[:ROWS_PER_CORE, 7 : 7 + N] = x_real[rows]
        shard[ROWS_PER_CORE:, 7 : 7 + N] = x_imag[rows]
        in_maps.append({"x": shard, "h4": h4, "ident": ident})
    nc = _get_nc()
    res = run_bass_kernel_spmd(nc, in_maps, core_ids=list(range(N_CORES)), trace=trace)
    out = np.empty((2, B, NOUT), np.float32)
    for c in range(N_CORES):
        yc = res.results[c]["y"]
        rows = slice(c * ROWS_PER_CORE, (c + 1) * ROWS_PER_CORE)
        out[0, rows] = yc[:ROWS_PER_CORE]
        out[1, rows] = yc[ROWS_PER_CORE:]
    return out, res


def kernel(x_real, x_imag, fir_filter, factor):
    assert int(factor) == FACTOR
    x_real = np.asarray(x_real, np.float32)
    x_imag = np.asarray(x_imag, np.float32)
    assert x_real.shape == (B, N) and x_imag.shape == (B, N)
    out, _ = _run(x_real, x_imag, fir_filter)
    return out


# revision 12
# speedup vs baseline: 1.2450x; 1.2450x over previous
"""Trainium2 Bass kernel for nn_Interpolator: zero-stuff upsample x8 + 128-tap FIR (SAME) + x8 gain.

Polyphase formulation: with m indexing 64-sample rows of x and n = 8*q' + r in [0, 512),
    y[512*m + n] = sum_{k=0}^{78} T4[k, m] * H4[k, n]
where T4[k, m] = x[64*m + k - 7] (zero-padded) and
    H4[k, 8*q'+r] = 8 * h[(7-r) + 8*(k-q')]  for 0 <= k-q' <= 15, else 0.

Per core (8 cores, batch-parallel): 16 signals (8 batch rows x {real, imag}).
Per signal: load host-padded x as [128, 271] fp16 (partition p = x[256p-7 : 256p+264]),
PE-transpose four 79-column slices into one PSUM bank, bulk-copy to T4 [79, 512] fp16
(columns interleaved m = 4p + c), then 4 matmuls lhsT=T4[:, 128t:+128], rhs=H4 -> PSUM
fp32 [128, 512], copy to SBUF, DMA out contiguously (partition i of tile t holds
y[65536t + 512i : +512]).  Matmul stage runs one signal behind the transpose stage so
the PE never waits on the PSUM->SBUF interleave copy.
"""

import numpy as np

import concourse.bass as bass
import concourse.tile as tile
from concourse import bacc, mybir
from concourse.bass_utils import run_bass_kernel_spmd

B = 64
N = 32768
FACTOR = 8
NOUT = N * FACTOR  # 262144
N_CORES = 8
ROWS_PER_CORE = B // N_CORES  # 8
SIGS = 2 * ROWS_PER_CORE  # 16 signals per core (real rows then imag rows)
K = 79  # contraction window length
XCOLS = 271  # 256 + 15 halo
NPAD = 32784  # 7 leading zeros + N + 8 trailing zeros + 1 spare (host-padded)
TILES = 4  # out tiles per signal, each [128 m-rows, 512 samples]

_F16 = mybir.dt.float16
_F32 = mybir.dt.float32

_NC_CACHE = {}


def _build_nc():
    nc = bacc.Bacc(
        "TRN2",
        target_bir_lowering=False,
        debug=False,
        enable_asserts=False,
        num_devices=N_CORES,
    )
    x = nc.dram_tensor("x", [SIGS, NPAD], _F16, kind="ExternalInput")
    h4 = nc.dram_tensor("h4", [K, 512], _F16, kind="ExternalInput")
    ident = nc.dram_tensor("ident", [128, 128], _F16, kind="ExternalInput")
    y = nc.dram_tensor("y", [SIGS, NOUT], _F32, kind="ExternalOutput")

    with tile.TileContext(nc) as tc:
        with (
            tc.tile_pool(name="consts", bufs=1) as consts,
            tc.tile_pool(name="xpool", bufs=2) as xpool,
            tc.tile_pool(name="t4pool", bufs=2) as t4pool,
            tc.tile_pool(name="opool", bufs=4) as opool,
            tc.tile_pool(name="pt", bufs=2, space="PSUM") as pt_pool,
            tc.tile_pool(name="po", bufs=3, space="PSUM") as po_pool,
        ):
            h4_sb = consts.tile([K, 512], _F16)
            nc.sync.dma_start(out=h4_sb, in_=h4.ap())
            ident_sb = consts.tile([128, 128], _F16)
            nc.sync.dma_start(out=ident_sb, in_=ident.ap())

            t4_tiles = [None] * SIGS

            def stage_a(sig):
                """Load + transpose + interleave-copy into T4."""
                xoff = sig * NPAD
                # partition p holds x_pad[256p : 256p + 271] = x[256p - 7 : 256p + 264]
                X = xpool.tile([128, XCOLS], _F16)
                nc.sync.dma_start(
                    out=X[:, :],
                    in_=bass.AP(tensor=x, offset=xoff, ap=[[256, 128], [1, XCOLS]]),
                )
                # All 4 transposes write one PSUM bank, one bulk copy out.
                # T4[k, 4p + c] = pt4[k, c, p] = X[p, 64c + k] = x[256p + 64c + k - 7]
                # Copy with permuted (strided) PSUM read + contiguous SBUF write:
                # strided fp16 SBUF writes pay a sub-word RMW penalty, reads don't.
                T4 = t4pool.tile([K, 512], _F16)
                pt4 = pt_pool.tile([K, 4, 128], _F16)
                for c in range(4):
                    nc.tensor.transpose(
                        pt4[:, c, :], X[:, 64 * c : 64 * c + K], ident_sb
                    )
                pt4_perm = pt4[:, :, :].rearrange("k c p -> k p c")
                if sig % 2 == 0:
                    nc.vector.tensor_copy(out=T4[:, :], in_=pt4_perm)
                else:
                    nc.scalar.copy(out=T4[:, :], in_=pt4_perm)
                t4_tiles[sig] = T4

            def stage_b(sig):
                """Matmuls + copy-out + store."""
                T4 = t4_tiles[sig]
                out_sb = opool.tile([128, TILES * 512], _F32)
                for half in range(2):
                    po = po_pool.tile([128, 1024], _F32)
                    for s in range(2):
                        t = 2 * half + s
                        nc.tensor.matmul(
                            po[:, 512 * s : 512 * (s + 1)],
                            T4[:, 128 * t : 128 * (t + 1)],
                            h4_sb[:, :],
                            start=True,
                            stop=True,
                        )
                    if half == 0:
                        nc.scalar.copy(out=out_sb[:, 0:1024], in_=po)
                    else:
                        nc.vector.tensor_copy(out=out_sb[:, 1024:2048], in_=po)
                # partition i, free (t, n) -> y[sig, 65536t + 512i + n]
                nc.gpsimd.dma_start(
                    out=bass.AP(
                        tensor=y,
                        offset=sig * NOUT,
                        ap=[[512, 128], [65536, TILES], [1, 512]],
                    ),
                    in_=out_sb[:, :],
                )

            for sig in range(SIGS):
                stage_a(sig)
                if sig >= 1:
                    stage_b(sig - 1)
            stage_b(SIGS - 1)

    nc.compile()
    return nc


def _get_nc():
    if "nc" not in _NC_CACHE:
        _NC_CACHE["nc"] = _build_nc()
    return _NC_CACHE["nc"]


def _build_h4(h):
    h4 = np.zeros((K, 512), np.float32)
    qp = np.arange(64)
    for t in range(16):
        for r in range(8):
            h4[qp + t, 8 * qp + r] = FACTOR * h[(7 - r) + 8 * t]
    return h4


def _run(x_real, x_imag, fir_filter, trace=False):
    h4 = _build_h4(np.asarray(fir_filter, np.float32)).astype(np.float16)
    ident = np.eye(128, dtype=np.float16)
    in_maps = []
    for c in range(N_CORES):
        rows = slice(c * ROWS_PER_CORE, (c + 1) * ROWS_PER_CORE)
        shard = np.zeros((SIGS, NPAD), np.float16)
        shard[:ROWS_PER_CORE, 7 : 7 + N] = x_real[rows]
        shard[ROWS_PER_CORE:, 7 : 7 + N] = x_imag[rows]
        in_maps.append({"x": shard, "h4": h4, "ident": ident})
    nc = _get_nc()
    res = run_bass_kernel_spmd(nc, in_maps, core_ids=list(range(N_CORES)), trace=trace)
    out = np.empty((2, B, NOUT), np.float32)
    for c in range(N_CORES):
        yc = res.results[c]["y"]
        rows = slice(c * ROWS_PER_CORE, (c + 1) * ROWS_PER_CORE)
        out[0, rows] = yc[:ROWS_PER_CORE]
        out[1, rows] = yc[ROWS_PER_CORE:]
    return out, res


def kernel(x_real, x_imag, fir_filter, factor):
    assert int(factor) == FACTOR
    x_real = np.asarray(x_real, np.float32)
    x_imag = np.asarray(x_imag, np.float32)
    assert x_real.shape == (B, N) and x_imag.shape == (B, N)
    out, _ = _run(x_real, x_imag, fir_filter)
    return out


# revision 14
# speedup vs baseline: 1.2465x; 1.0012x over previous
"""Trainium2 Bass kernel for nn_Interpolator: zero-stuff upsample x8 + 128-tap FIR (SAME) + x8 gain.

Polyphase formulation: with m indexing 64-sample rows of x and n = 8*q' + r in [0, 512),
    y[512*m + n] = sum_{k=0}^{78} T4[k, m] * H4[k, n]
where T4[k, m] = x[64*m + k - 7] (zero-padded) and
    H4[k, 8*q'+r] = 8 * h[(7-r) + 8*(k-q')]  for 0 <= k-q' <= 15, else 0.

Per core (8 cores, batch-parallel): 16 signals (8 batch rows x {real, imag}).
Per signal: load host-padded x as [128, 271] fp16 (partition p = x[256p-7 : 256p+264]),
PE-transpose four 79-column slices into one PSUM bank, bulk-copy to T4 [79, 512] fp16
(columns interleaved m = 4p + c), then 4 matmuls lhsT=T4[:, 128t:+128], rhs=H4 -> PSUM
fp32 [128, 512], copy to SBUF, DMA out contiguously (partition i of tile t holds
y[65536t + 512i : +512]).  Matmul stage runs one signal behind the transpose stage so
the PE never waits on the PSUM->SBUF interleave copy.
"""

import numpy as np

import concourse.bass as bass
import concourse.tile as tile
from concourse import bacc, mybir
from concourse.bass_utils import run_bass_kernel_spmd

B = 64
N = 32768
FACTOR = 8
NOUT = N * FACTOR  # 262144
N_CORES = 8
ROWS_PER_CORE = B // N_CORES  # 8
SIGS = 2 * ROWS_PER_CORE  # 16 signals per core (real rows then imag rows)
K = 79  # contraction window length
XCOLS = 271  # 256 + 15 halo
NPAD = 32784  # 7 leading zeros + N + 8 trailing zeros + 1 spare (host-padded)
TILES = 4  # out tiles per signal, each [128 m-rows, 512 samples]

_F16 = mybir.dt.float16
_F32 = mybir.dt.float32

_NC_CACHE = {}


def _build_nc():
    nc = bacc.Bacc(
        "TRN2",
        target_bir_lowering=False,
        debug=False,
        enable_asserts=False,
        num_devices=N_CORES,
    )
    x = nc.dram_tensor("x", [SIGS, NPAD], _F16, kind="ExternalInput")
    h4 = nc.dram_tensor("h4", [K, 512], _F16, kind="ExternalInput")
    ident = nc.dram_tensor("ident", [128, 128], _F16, kind="ExternalInput")
    y = nc.dram_tensor("y", [SIGS, NOUT], _F32, kind="ExternalOutput")

    with tile.TileContext(nc) as tc:
        with (
            tc.tile_pool(name="consts", bufs=1) as consts,
            tc.tile_pool(name="xpool", bufs=6) as xpool,
            tc.tile_pool(name="t4pool", bufs=8) as t4pool,
            tc.tile_pool(name="opool", bufs=6) as opool,
            tc.tile_pool(name="pt", bufs=3, space="PSUM") as pt_pool,
            tc.tile_pool(name="po", bufs=2, space="PSUM") as po_pool,
        ):
            h4_sb = consts.tile([K, 512], _F16)
            nc.sync.dma_start(out=h4_sb, in_=h4.ap())
            ident_sb = consts.tile([128, 128], _F16)
            nc.sync.dma_start(out=ident_sb, in_=ident.ap())

            t4_tiles = [None] * SIGS

            def stage_a(sig):
                """Load + transpose + interleave-copy into T4."""
                xoff = sig * NPAD
                # partition p holds x_pad[256p : 256p + 271] = x[256p - 7 : 256p + 264]
                X = xpool.tile([128, XCOLS], _F16)
                nc.sync.dma_start(
                    out=X[:, :],
                    in_=bass.AP(tensor=x, offset=xoff, ap=[[256, 128], [1, XCOLS]]),
                )
                # All 4 transposes write one PSUM bank, one bulk copy out.
                # T4[k, 4p + c] = pt4[k, c, p] = X[p, 64c + k] = x[256p + 64c + k - 7]
                # Copy with permuted (strided) PSUM read + contiguous SBUF write:
                # strided fp16 SBUF writes pay a sub-word RMW penalty, reads don't.
                T4 = t4pool.tile([K, 512], _F16)
                pt4 = pt_pool.tile([K, 4, 128], _F16)
                for c in range(4):
                    nc.tensor.transpose(
                        pt4[:, c, :], X[:, 64 * c : 64 * c + K], ident_sb
                    )
                pt4_perm = pt4[:, :, :].rearrange("k c p -> k p c")
                if sig % 2 == 0:
                    nc.vector.tensor_copy(out=T4[:, :], in_=pt4_perm)
                else:
                    nc.scalar.copy(out=T4[:, :], in_=pt4_perm)
                t4_tiles[sig] = T4

            def stage_b(sig):
                """Matmuls + copy-out + store."""
                T4 = t4_tiles[sig]
                out_sb = opool.tile([128, TILES * 512], _F32)
                for half in range(2):
                    po = po_pool.tile([128, 1024], _F32)
                    for s in range(2):
                        t = 2 * half + s
                        nc.tensor.matmul(
                            po[:, 512 * s : 512 * (s + 1)],
                            T4[:, 128 * t : 128 * (t + 1)],
                            h4_sb[:, :],
                            start=True,
                            stop=True,
                        )
                    if half == 0:
                        nc.scalar.copy(out=out_sb[:, 0:1024], in_=po)
                    else:
                        nc.vector.tensor_copy(out=out_sb[:, 1024:2048], in_=po)
                # partition i, free (t, n) -> y[sig, 65536t + 512i + n]
                nc.gpsimd.dma_start(
                    out=bass.AP(
                        tensor=y,
                        offset=sig * NOUT,
                        ap=[[512, 128], [65536, TILES], [1, 512]],
                    ),
                    in_=out_sb[:, :],
                )

            # Batch 4 signals per stage: 16 back-to-back matmuls per stage_b
            # keep the PE busy long enough for HAM to unthrottle to 2.4 GHz
            # (lone 4-matmul bursts never warm it).
            BATCH = 4
            for b in range(SIGS // BATCH):
                for s in range(BATCH):
                    stage_a(BATCH * b + s)
                if b >= 1:
                    for s in range(BATCH):
                        stage_b(BATCH * (b - 1) + s)
            for s in range(BATCH):
                stage_b(SIGS - BATCH + s)

    nc.compile()
    return nc


def _get_nc():
    if "nc" not in _NC_CACHE:
        _NC_CACHE["nc"] = _build_nc()
    return _NC_CACHE["nc"]


def _build_h4(h):
    h4 = np.zeros((K, 512), np.float32)
    qp = np.arange(64)
    for t in range(16):
        for r in range(8):
            h4[qp + t, 8 * qp + r] = FACTOR * h[(7 - r) + 8 * t]
    return h4


def _run(x_real, x_imag, fir_filter, trace=False):
    h4 = _build_h4(np.asarray(fir_filter, np.float32)).astype(np.float16)
    ident = np.eye(128, dtype=np.float16)
    in_maps = []
    for c in range(N_CORES):
        rows = slice(c * ROWS_PER_CORE, (c + 1) * ROWS_PER_CORE)
        shard = np.zeros((SIGS, NPAD), np.float16)
        shard[:ROWS_PER_CORE, 7 : 7 + N] = x_real[rows]
        shard[ROWS_PER_CORE:, 7 : 7 + N] = x_imag[rows]
        in_maps.append({"x": shard, "h4": h4, "ident": ident})
    nc = _get_nc()
    res = run_bass_kernel_spmd(nc, in_maps, core_ids=list(range(N_CORES)), trace=trace)
    out = np.empty((2, B, NOUT), np.float32)
    for c in range(N_CORES):
        yc = res.results[c]["y"]
        rows = slice(c * ROWS_PER_CORE, (c + 1) * ROWS_PER_CORE)
        out[0, rows] = yc[:ROWS_PER_CORE]
        out[1, rows] = yc[ROWS_PER_CORE:]
    return out, res


def kernel(x_real, x_imag, fir_filter, factor):
    assert int(factor) == FACTOR
    x_real = np.asarray(x_real, np.float32)
    x_imag = np.asarray(x_imag, np.float32)
    assert x_real.shape == (B, N) and x_imag.shape == (B, N)
    out, _ = _run(x_real, x_imag, fir_filter)
    return out


# revision 15
# speedup vs baseline: 1.2886x; 1.0338x over previous
"""Trainium2 Bass kernel for nn_Interpolator: zero-stuff upsample x8 + 128-tap FIR (SAME) + x8 gain.

Polyphase formulation: with m indexing 64-sample rows of x and n = 8*q' + r in [0, 512),
    y[512*m + n] = sum_{k=0}^{78} T4[k, m] * H4[k, n]
where T4[k, m] = x[64*m + k - 7] (zero-padded) and
    H4[k, 8*q'+r] = 8 * h[(7-r) + 8*(k-q')]  for 0 <= k-q' <= 15, else 0.

Per core (8 cores, batch-parallel): 16 signals (8 batch rows x {real, imag}).
Per signal: load host-padded x as [128, 271] fp16 (partition p = x[256p-7 : 256p+264]),
PE-transpose four 79-column slices into one PSUM bank, bulk-copy to T4 [79, 512] fp16
(columns interleaved m = 4p + c), then 4 matmuls lhsT=T4[:, 128t:+128], rhs=H4 -> PSUM
fp32 [128, 512], copy to SBUF, DMA out contiguously (partition i of tile t holds
y[65536t + 512i : +512]).  Matmul stage runs one signal behind the transpose stage so
the PE never waits on the PSUM->SBUF interleave copy.
"""

import numpy as np

import concourse.bass as bass
import concourse.tile as tile
from concourse import bacc, mybir
from concourse.bass_utils import run_bass_kernel_spmd

B = 64
N = 32768
FACTOR = 8
NOUT = N * FACTOR  # 262144
N_CORES = 8
ROWS_PER_CORE = B // N_CORES  # 8
SIGS = 2 * ROWS_PER_CORE  # 16 signals per core (real rows then imag rows)
K = 79  # contraction window length
XCOLS = 271  # 256 + 15 halo
NPAD = 32784  # 7 leading zeros + N + 8 trailing zeros + 1 spare (host-padded)
TILES = 4  # out tiles per signal, each [128 m-rows, 512 samples]

_F16 = mybir.dt.float16
_F32 = mybir.dt.float32

_NC_CACHE = {}


def _build_nc():
    nc = bacc.Bacc(
        "TRN2",
        target_bir_lowering=False,
        debug=False,
        enable_asserts=False,
        num_devices=N_CORES,
    )
    x = nc.dram_tensor("x", [SIGS, NPAD], _F16, kind="ExternalInput")
    h4 = nc.dram_tensor("h4", [K, 512], _F16, kind="ExternalInput")
    ident = nc.dram_tensor("ident", [128, 128], _F16, kind="ExternalInput")
    y = nc.dram_tensor("y", [SIGS, NOUT], _F32, kind="ExternalOutput")

    with tile.TileContext(nc) as tc:
        with (
            tc.tile_pool(name="consts", bufs=1) as consts,
            tc.tile_pool(name="xpool", bufs=6) as xpool,
            tc.tile_pool(name="t4pool", bufs=8) as t4pool,
            tc.tile_pool(name="opool", bufs=6) as opool,
            tc.tile_pool(name="pt", bufs=3, space="PSUM") as pt_pool,
            tc.tile_pool(name="po", bufs=2, space="PSUM") as po_pool,
        ):
            h4_sb = consts.tile([K, 512], _F16)
            nc.sync.dma_start(out=h4_sb, in_=h4.ap())
            ident_sb = consts.tile([128, 128], _F16)
            nc.sync.dma_start(out=ident_sb, in_=ident.ap())

            t4_tiles = [None] * SIGS

            def stage_a(sig):
                """Load + transpose + interleave-copy into T4."""
                xoff = sig * NPAD
                # partition p holds x_pad[256p : 256p + 271] = x[256p - 7 : 256p + 264]
                X = xpool.tile([128, XCOLS], _F16)
                nc.sync.dma_start(
                    out=X[:, :],
                    in_=bass.AP(tensor=x, offset=xoff, ap=[[256, 128], [1, XCOLS]]),
                )
                # All 4 transposes write one PSUM bank, one bulk copy out.
                # T4[k, 4p + c] = pt4[k, c, p] = X[p, 64c + k] = x[256p + 64c + k - 7]
                # Copy with permuted (strided) PSUM read + contiguous SBUF write:
                # strided fp16 SBUF writes pay a sub-word RMW penalty, reads don't.
                T4 = t4pool.tile([K, 512], _F16)
                pt4 = pt_pool.tile([K, 4, 128], _F16)
                for c in range(4):
                    nc.tensor.transpose(
                        pt4[:, c, :], X[:, 64 * c : 64 * c + K], ident_sb
                    )
                pt4_perm = pt4[:, :, :].rearrange("k c p -> k p c")
                if sig % 2 == 0:
                    nc.vector.tensor_copy(out=T4[:, :], in_=pt4_perm)
                else:
                    nc.scalar.copy(out=T4[:, :], in_=pt4_perm)
                t4_tiles[sig] = T4

            def stage_b(sig):
                """Matmuls + copy-out + store (two independent 512 KB halves)."""
                T4 = t4_tiles[sig]
                out_sb = opool.tile([128, TILES * 512], _F32)
                for half in range(2):
                    po = po_pool.tile([128, 1024], _F32)
                    for s in range(2):
                        t = 2 * half + s
                        nc.tensor.matmul(
                            po[:, 512 * s : 512 * (s + 1)],
                            T4[:, 128 * t : 128 * (t + 1)],
                            h4_sb[:, :],
                            start=True,
                            stop=True,
                        )
                    if half == 0:
                        nc.scalar.copy(out=out_sb[:, 0:1024], in_=po)
                    else:
                        nc.vector.tensor_copy(out=out_sb[:, 1024:2048], in_=po)
                    # partition i, free (t, n) -> y[sig, 65536t + 512i + n]
                    nc.sync.dma_start(
                        out=bass.AP(
                            tensor=y,
                            offset=sig * NOUT + half * 2 * 65536,
                            ap=[[512, 128], [65536, 2], [1, 512]],
                        ),
                        in_=out_sb[:, 1024 * half : 1024 * (half + 1)],
                    )

            # Batch 4 signals per stage: 16 back-to-back matmuls per stage_b
            # keep the PE busy long enough for HAM to unthrottle to 2.4 GHz
            # (lone 4-matmul bursts never warm it).
            BATCH = 4
            for b in range(SIGS // BATCH):
                for s in range(BATCH):
                    stage_a(BATCH * b + s)
                if b >= 1:
                    for s in range(BATCH):
                        stage_b(BATCH * (b - 1) + s)
            for s in range(BATCH):
                stage_b(SIGS - BATCH + s)

    nc.compile()
    return nc


def _get_nc():
    if "nc" not in _NC_CACHE:
        _NC_CACHE["nc"] = _build_nc()
    return _NC_CACHE["nc"]


def _build_h4(h):
    h4 = np.zeros((K, 512), np.float32)
    qp = np.arange(64)
    for t in range(16):
        for r in range(8):
            h4[qp + t, 8 * qp + r] = FACTOR * h[(7 - r) + 8 * t]
    return h4


def _run(x_real, x_imag, fir_filter, trace=False):
    h4 = _build_h4(np.asarray(fir_filter, np.float32)).astype(np.float16)
    ident = np.eye(128, dtype=np.float16)
    in_maps = []
    for c in range(N_CORES):
        rows = slice(c * ROWS_PER_CORE, (c + 1) * ROWS_PER_CORE)
        shard = np.zeros((SIGS, NPAD), np.float16)
        shard[:ROWS_PER_CORE, 7 : 7 + N] = x_real[rows]
        shard[ROWS_PER_CORE:, 7 : 7 + N] = x_imag[rows]
        in_maps.append({"x": shard, "h4": h4, "ident": ident})
    nc = _get_nc()
    res = run_bass_kernel_spmd(nc, in_maps, core_ids=list(range(N_CORES)), trace=trace)
    out = np.empty((2, B, NOUT), np.float32)
    for c in range(N_CORES):
        yc = res.results[c]["y"]
        rows = slice(c * ROWS_PER_CORE, (c + 1) * ROWS_PER_CORE)
        out[0, rows] = yc[:ROWS_PER_CORE]
        out[1, rows] = yc[ROWS_PER_CORE:]
    return out, res


def kernel(x_real, x_imag, fir_filter, factor):
    assert int(factor) == FACTOR
    x_real = np.asarray(x_real, np.float32)
    x_imag = np.asarray(x_imag, np.float32)
    assert x_real.shape == (B, N) and x_imag.shape == (B, N)
    out, _ = _run(x_real, x_imag, fir_filter)
    return out
